# revision 50
# baseline (speedup 1.0000x reference)
"""Fused MLA-with-GQA attention kernel for 8 Trainium2 NeuronCores.

Sharding: 8 cores = 2 (batch) x 4 (kv-head groups); core = 4*b + g owns
batch b, query heads 4g..4g+3 and kv head g.

Host->device traffic is minimized: every input byte is uploaded exactly
once (bf16), and on-device AllGathers over the chip interconnect rebuild
the per-core operands:
  xs   [512,2048]  X rows owned by this core      -> AllGather {b-group}
  w1q  [1024,768]  half of group-g Wqkv q-cols    -> AllGather {pair}
  lra  [256,512]   1/8 of the shared LoRA cols    -> AllGather {all 8}
  wk/wv [256,*]    half of group-g up-projections -> AllGather {pair}
  wo   [256,2048]  quarter... half of g-rows of Wo-> AllGather {pair}
Rope cos/sin tables, the triangular mask and the transpose identity are
compile-time constants embedded in the NEFF. The partial outputs are
ReduceScattered per s-chunk across each batch group, so every core
downloads only its own [512, 2048] bf16 slice of Y.

On-device layout is fully transposed (feature-major); X^T is produced
on-device with PE transposes of the gathered X. All matmul operands are
bf16 (full-rate), accumulation in fp32 PSUM. Causal structure: k-tiles
above the diagonal are skipped; diagonal k-tiles compute the column
sub-range [p:512] only, with a triangular mask multiply after exp.
"""

import math
import sys

import numpy as np

for _p in ("/opt/trn_rl_repo", "/root/.axon_site/_ro/trn_rl_repo"):
    if _p not in sys.path:
        try:
            import os

            if os.path.isdir(_p):
                sys.path.insert(0, _p)
        except Exception:
            pass

import ml_dtypes

import concourse.bacc as bacc
import concourse.mybir as mybir
import concourse.tile as tile
from concourse.alu_op_type import AluOpType
from concourse.bass_utils import run_bass_kernel_spmd

BF16 = ml_dtypes.bfloat16

# ---- problem constants (hardcoded; kernel.py must be self-contained) ----
HID = 2048
NH = 16
NKV = 4
NG = NH // NKV  # 4 q heads per kv head
LORA = 512
D_ROPE = 64
D_NOPE = 128
D_V = 128
D_QK = D_NOPE + D_ROPE  # 192
B, S = 2, 2048
ROPE_BASE = 10000.0
NCORES = 8

NHC = NG  # heads per core = 4
SC = 512  # s-chunk width
NCHUNK = S // SC  # 4
KT = 128  # k tile
NKT_TOT = S // KT  # 16
SCALE = 1.0 / math.sqrt(D_QK)
QCOLS = NHC * D_QK  # 768 packed q columns per group

# packed per-core weight buffer (bf16 elements): halves of w1q, lra, wk,
# wv, wo concatenated flat; pair-AllGather rebuilds both halves.
OFF_W1Q = 0
OFF_LRA = OFF_W1Q + (HID // 2) * QCOLS      # 786432
OFF_WK = OFF_LRA + (HID // 2) * (LORA // 4)  # 917504
OFF_WV = OFF_WK + (LORA // 2) * D_QK         # 966656
OFF_WO = OFF_WV + (LORA // 2) * D_V          # 999424
WFLAT = OFF_WO + (NHC * D_V // 2) * HID      # 1523712

G_BATCH = [[0, 1, 2, 3], [4, 5, 6, 7]]
G_PAIR = [[0, 4], [1, 5], [2, 6], [3, 7]]
G_ALL = [[0, 1, 2, 3, 4, 5, 6, 7]]

F32 = mybir.dt.float32
BF = mybir.dt.bfloat16
EXP = mybir.ActivationFunctionType.Exp

_PROGRAM_CACHE = {}


def _rope_tables():
    inv_freq = 1.0 / (ROPE_BASE ** (np.arange(0, D_ROPE, 2, dtype=np.float64) / D_ROPE))
    t = np.arange(S, dtype=np.float64)
    freqs = np.outer(t, inv_freq)  # [S, 32]
    return np.cos(freqs).T, np.sin(freqs).T  # each [32, S]


def _build_program(reps: int = 1, single: bool = False):
    """single=True replaces collectives with local DMAs (for CoreSim timing
    only; numerics are wrong in that mode)."""
    nc = bacc.Bacc("TRN2", target_bir_lowering=False, debug=False, num_devices=NCORES)

    # ---- per-core uploads (bf16) ----
    xs_d = nc.dram_tensor("xs", [SC, HID], BF, kind="ExternalInput").ap()
    wpk_d = nc.dram_tensor("wpk", [WFLAT], BF, kind="ExternalInput").ap()
    y_d = nc.dram_tensor("y", [SC, HID], BF, kind="ExternalOutput").ap()

    # ---- compile-time constants embedded in the NEFF ----
    cosT, sinT = _rope_tables()
    cs_d = nc.inline_tensor(
        np.concatenate([cosT, sinT], axis=0).astype(BF16), name="cs"
    ).ap()  # [64, S]
    tri_d = nc.inline_tensor(
        np.triu(np.ones((128, 128))).astype(BF16), name="trimask"
    ).ap()
    eye_d = nc.inline_tensor(np.eye(128).astype(BF16), name="eye128").ap()

    # ---- internal DRAM: collective bounces and gathered operands ----
    wpk_b = nc.dram_tensor("wpk_b", [WFLAT], BF, kind="Internal").ap()
    wpkg = nc.dram_tensor("wpkg", [2 * WFLAT], BF, kind="Internal").ap()
    ckv_b = nc.dram_tensor("ckv_b", [LORA // 4, SC], BF, kind="Internal").ap()
    xt_b = nc.dram_tensor("xt_b", [HID, SC], BF, kind="Internal").ap()
    xtg = nc.dram_tensor("xtg", [NCHUNK * HID, SC], BF, kind="Internal").ap()
    ckvg = nc.dram_tensor("ckvg", [LORA, SC], BF, kind="Internal").ap()

    def wview(b, off, lr, rows, cols):
        base = b * WFLAT + off + lr * cols
        return wpkg[base : base + rows * cols].rearrange(
            "(r c) -> r c", r=rows, c=cols
        )
    yp = nc.dram_tensor("yp", [S, HID], BF, kind="Internal").ap()
    yrs = nc.dram_tensor("yrs", [SC, HID], BF, kind="Internal").ap()

    from contextlib import ExitStack

    with tile.TileContext(nc) as tc:
        with ExitStack() as ctx:
            constp = ctx.enter_context(tc.tile_pool(name="const", bufs=1))
            stagep = ctx.enter_context(tc.tile_pool(name="stage", bufs=1))
            wop = ctx.enter_context(tc.tile_pool(name="wo", bufs=1))
            w1p = ctx.enter_context(tc.tile_pool(name="w1s", bufs=1))
            xrp = ctx.enter_context(tc.tile_pool(name="xr", bufs=1))
            xp = ctx.enter_context(tc.tile_pool(name="x", bufs=1))
            qnp = ctx.enter_context(tc.tile_pool(name="qn", bufs=1))
            ckvp = ctx.enter_context(tc.tile_pool(name="ckv", bufs=1))
            kfp = ctx.enter_context(tc.tile_pool(name="kf", bufs=1))
            vp = ctx.enter_context(tc.tile_pool(name="v", bufs=1))
            ropep = ctx.enter_context(tc.tile_pool(name="rope", bufs=1))
            ep = ctx.enter_context(tc.tile_pool(name="e", bufs=3))
            onp = ctx.enter_context(tc.tile_pool(name="on", bufs=1))
            yp_sb = ctx.enter_context(tc.tile_pool(name="y", bufs=2))
            mmp = ctx.enter_context(tc.tile_pool(name="mm", bufs=6, space="PSUM"))
            denp = ctx.enter_context(tc.tile_pool(name="den", bufs=1, space="PSUM"))
            op_ = ctx.enter_context(tc.tile_pool(name="o", bufs=1, space="PSUM"))

            # ---------------- constants ----------------
            tri_t = constp.tile([128, 128], BF, tag="tri")
            nc.scalar.dma_start(tri_t[:], tri_d[:])
            eye_t = constp.tile([128, 128], BF, tag="eye")
            nc.sync.dma_start(eye_t[:], eye_d[:])

            ones_f = constp.tile([128, 128], F32, tag="ones_f")
            nc.gpsimd.memset(ones_f[:], 1.0)
            ones_b = constp.tile([128, 128], BF, tag="ones_b")
            nc.scalar.copy(ones_b[:], ones_f[:])

            # full-S rope tables, 4 head-copies of 32 rows each
            cos_full = constp.tile([128, S], BF, tag="cosf")
            sin_full = constp.tile([128, S], BF, tag="sinf")
            for h in range(4):
                nc.scalar.dma_start(cos_full[32 * h : 32 * h + 32, :], cs_d[0:32, :])
                nc.sync.dma_start(sin_full[32 * h : 32 * h + 32, :], cs_d[32:64, :])

            def stage(bounce, src, rows, cols, tag):
                """ExternalInput -> SBUF -> internal DRAM bounce."""
                for i, r0 in enumerate(range(0, rows, 128)):
                    r1 = min(r0 + 128, rows)
                    t = stagep.tile([128, cols], BF, tag=tag, name=f"{tag}_{r0}")
                    reng = nc.sync if i % 2 == 0 else nc.scalar
                    reng.dma_start(t[0 : r1 - r0, :], src[r0:r1, :])
                    nc.gpsimd.dma_start(bounce[r0:r1, :], t[0 : r1 - r0, :])

            def gather(kind_groups, in_ap, out_ap, nsplit):
                if single:
                    # timing-only: copy own contribution to slot 0
                    rows = in_ap.shape[0]
                    for r0 in range(0, rows, 128):
                        r1 = min(r0 + 128, rows)
                        t = stagep.tile(
                            [128, in_ap.shape[1]], BF,
                            tag=f"agt_{out_ap.tensor.name}",
                            name=f"agt_{out_ap.tensor.name}_{r0}",
                        )
                        nc.sync.dma_start(t[0 : r1 - r0, :], in_ap[r0:r1, :])
                        nc.gpsimd.dma_start(out_ap[r0:r1, :], t[0 : r1 - r0, :])
                else:
                    nc.gpsimd.collective_compute(
                        "AllGather",
                        mybir.AluOpType.bypass,
                        replica_groups=kind_groups,
                        ins=[in_ap[:]],
                        outs=[out_ap[:]],
                    )

            # persistent K / V state across chunks
            k_nope = kfp.tile([128, S], BF, tag="k_nope")
            k_rope = kfp.tile([64, S], BF, tag="k_rope")
            v_t = [
                vp.tile([128, D_V], BF, tag=f"v{i}", name=f"v{i}")
                for i in range(NKT_TOT)
            ]

            for rep in range(reps):
                # ---- transpose own X slice, stage bounces, then gather ----
                # The own-slice transposes run before any collective; the
                # gathered tensor is already feature-major per source chunk.
                xo_t = [
                    xp.tile([128, SC], BF, tag=f"xo{ht}", name=f"xo{ht}_{rep}")
                    for ht in range(16)
                ]
                for k in range(4):
                    xsr = xrp.tile([128, HID], BF, tag="xrow", bufs=2,
                                   name=f"xsr_{k}_{rep}")
                    nc.sync.dma_start(xsr[:], xs_d[128 * k : 128 * (k + 1), :])
                    for ht in range(16):
                        ps_t = mmp.tile([128, 128], BF, tag="mm")
                        nc.tensor.transpose(
                            ps_t[:], xsr[:, 128 * ht : 128 * (ht + 1)], eye_t[:]
                        )
                        nc.vector.tensor_copy(
                            xo_t[ht][:, 128 * k : 128 * (k + 1)], ps_t[:]
                        )
                        if k == 3:
                            # tile ht is complete - ship it while later ht
                            # transposes are still running
                            weng = nc.gpsimd if ht % 2 == 0 else nc.scalar
                            weng.dma_start(
                                xt_b[128 * ht : 128 * (ht + 1), :], xo_t[ht][:]
                            )

                # gathers are a serial stream: X first (it gates the first
                # compute), then the single packed-weights pair-gather.
                PCOL = 1984  # WFLAT = 128 * 11904 = 128 * 6 * 1984
                for i in range(6):
                    o0 = i * 128 * PCOL
                    t = stagep.tile([128, PCOL], BF, tag="sg_wpk",
                                    name=f"sg_wpk_{i}_{rep}")
                    reng = nc.sync if i % 2 == 0 else nc.scalar
                    reng.dma_start(
                        t[:],
                        wpk_d[o0 : o0 + 128 * PCOL].rearrange(
                            "(r c) -> r c", r=128, c=PCOL
                        ),
                    )
                    nc.gpsimd.dma_start(
                        wpk_b[o0 : o0 + 128 * PCOL].rearrange(
                            "(r c) -> r c", r=128, c=PCOL
                        ),
                        t[:],
                    )
                gather(G_BATCH, xt_b, xtg, 4)
                if single:
                    for i in range(6):
                        o0 = i * 128 * PCOL
                        t = stagep.tile([128, PCOL], BF, tag="sg_wpk",
                                        name=f"agt_wpk_{i}_{rep}")
                        nc.sync.dma_start(
                            t[:],
                            wpk_b[o0 : o0 + 128 * PCOL].rearrange(
                                "(r c) -> r c", r=128, c=PCOL
                            ),
                        )
                        nc.gpsimd.dma_start(
                            wpkg[o0 : o0 + 128 * PCOL].rearrange(
                                "(r c) -> r c", r=128, c=PCOL
                            ),
                            t[:],
                        )
                else:
                    nc.gpsimd.collective_compute(
                        "AllGather",
                        mybir.AluOpType.bypass,
                        replica_groups=G_PAIR,
                        ins=[wpk_b[:]],
                        outs=[wpkg[:]],
                    )

                # wk/wv l-tiles
                wk_t = []
                wv_t = []
                for l in range(4):
                    b_, lr = l // 2, 128 * l - 256 * (l // 2)
                    t = constp.tile([128, D_QK], BF, tag=f"wk{l}",
                                    name=f"wk{l}_{rep}")
                    nc.gpsimd.dma_start(t[:], wview(b_, OFF_WK, lr, 128, D_QK))
                    wk_t.append(t)
                    t = constp.tile([128, D_V], BF, tag=f"wv{l}",
                                    name=f"wv{l}_{rep}")
                    nc.gpsimd.dma_start(t[:], wview(b_, OFF_WV, lr, 128, D_V))
                    wv_t.append(t)

                # wo resident: per (head, n-block) tiles [128, 512]
                wo_t = [[None] * 4 for _ in range(NHC)]
                for h in range(NHC):
                    for n in range(4):
                        t = wop.tile([128, 512], BF, tag=f"wo{h}_{n}",
                                     name=f"wo{h}_{n}_{rep}")
                        b_, lr = h // 2, 128 * h - 256 * (h // 2)
                        nc.gpsimd.dma_start(
                            t[:],
                            wview(b_, OFF_WO, lr, 128, HID)[
                                :, 512 * n : 512 * (n + 1)
                            ],
                        )
                        wo_t[h][n] = t

                # ---- resident W1 (q-cols + own LoRA block), loaded once ----
                w1_t = {}
                wl_t = {}
                # wl (LoRA) tiles first: j=6 is the first matmul group and
                # needs only these 0.5MB, not the 3MB of w1 q-columns
                for ht in range(16):
                    b_, lr = ht // 8, 128 * ht - 1024 * (ht // 8)
                    t2 = w1p.tile([128, 128], BF, tag=f"wl_{ht}",
                                  name=f"wl_{ht}_{rep}")
                    eng = nc.scalar if ht % 2 == 0 else nc.sync
                    eng.dma_start(t2[:], wview(b_, OFF_LRA, lr, 128, LORA // 4))
                    wl_t[ht] = t2
                for ht in range(16):
                    b_, lr = ht // 8, 128 * ht - 1024 * (ht // 8)
                    t = w1p.tile([128, QCOLS], BF, tag=f"w1_{ht}",
                                 name=f"w1_{ht}_{rep}")
                    eng = nc.scalar if ht % 2 == 0 else nc.sync
                    eng.dma_start(t[:], wview(b_, OFF_W1Q, lr, 128, QCOLS))
                    w1_t[ht] = t

                def pend_rs(c):
                    s0 = SC * c
                    if single:
                        t = stagep.tile([128, HID], BF, tag="rst",
                                        name=f"rst_{c}_{rep}")
                        nc.sync.dma_start(t[:], yp[s0 : s0 + 128, :])
                        nc.gpsimd.dma_start(yrs[128 * c : 128 * (c + 1), :], t[:])
                    else:
                        nc.gpsimd.collective_compute(
                            "ReduceScatter",
                            mybir.AluOpType.add,
                            replica_groups=G_BATCH,
                            ins=[yp[s0 : s0 + SC, :]],
                            outs=[yrs[128 * c : 128 * (c + 1), :]],
                        )
                    # copy the scattered piece to the output tensor
                    t = stagep.tile([128, HID], BF, tag="sgbig",
                                    name=f"yout_{c}_{rep}")
                    nc.sync.dma_start(t[:], yrs[128 * c : 128 * (c + 1), :])
                    nc.gpsimd.dma_start(y_d[128 * c : 128 * (c + 1), :], t[:])

                for c in range(NCHUNK):
                    s0 = SC * c

                    # x_t tiles for this chunk straight from the gathered X^T
                    x_t = []
                    for ht in range(16):
                        t = xp.tile([128, SC], BF, tag=f"x{ht}", bufs=2,
                                    name=f"x{ht}_{c}_{rep}")
                        eng = nc.sync if ht % 2 == 0 else nc.scalar
                        eng.dma_start(
                            t[:],
                            xtg[HID * c + 128 * ht : HID * c + 128 * (ht + 1), :],
                        )
                        x_t.append(t)

                    # ------- phase A: C1 = X @ W1 (transposed) -------
                    # q columns (6 j's) + this core's 128 LoRA features (1 j);
                    # the other 3 LoRA blocks come from the batch-group
                    # AllGather of ckv below.
                    q_nope = []
                    qx1_ps = qx2_ps = None
                    for j in (6, 0, 1, 2, 3, 4, 5):
                        ps = mmp.tile([128, SC], F32, tag="mm")
                        for ht in range(16):
                            lhsT = (
                                w1_t[ht][:, 128 * j : 128 * (j + 1)]
                                if j < 6
                                else wl_t[ht][:]
                            )
                            nc.tensor.matmul(
                                ps[:],
                                lhsT,
                                x_t[ht][:],
                                start=(ht == 0),
                                stop=(ht == 15),
                            )
                        if j < 4:
                            t = qnp.tile([128, SC], BF, tag=f"qn{j}")
                            nc.scalar.copy(t[:], ps[:])
                            q_nope.append(t)
                        elif j == 4:
                            qx1_ps = ps
                        elif j == 5:
                            qx2_ps = ps
                        else:
                            ckv_own = ckvp.tile([128, SC], BF, tag="ckv_own")
                            nc.scalar.copy(ckv_own[:], ps[:])
                            nc.gpsimd.dma_start(ckv_b[:], ckv_own[:])
                            # gather the 4 LoRA blocks of c_kv across the
                            # batch group; hidden under the remaining q j's
                            if single:
                                t = stagep.tile([128, SC], BF, tag="agt_ckvg",
                                                name=f"agt_ckvg_{c}_{rep}")
                                nc.sync.dma_start(t[:], ckv_b[:])
                                nc.gpsimd.dma_start(ckvg[0:128, :], t[:])
                            else:
                                nc.gpsimd.collective_compute(
                                    "AllGather",
                                    mybir.AluOpType.bypass,
                                    replica_groups=G_BATCH,
                                    ins=[ckv_b[:]],
                                    outs=[ckvg[:]],
                                )
                            # deferred ReduceScatter of the previous chunk's
                            # Y (keeps the ckv AllGather ahead in the stream)
                            if c > 0:
                                pend_rs(c - 1)
                    cos_t = cos_full[:, s0 : s0 + SC]
                    sin_t = sin_full[:, s0 : s0 + SC]

                    # ---- Q rope (4 heads batched in 128 partitions) ----
                    p1 = ropep.tile([128, SC], F32, tag="p1")
                    t1 = ropep.tile([128, SC], F32, tag="t1")
                    p2 = ropep.tile([128, SC], F32, tag="p2")
                    t2 = ropep.tile([128, SC], F32, tag="t2")
                    nc.vector.tensor_tensor(p1[:], qx1_ps[:], cos_t, AluOpType.mult)
                    nc.vector.tensor_tensor(t1[:], qx2_ps[:], sin_t, AluOpType.mult)
                    nc.vector.tensor_tensor(p2[:], qx2_ps[:], cos_t, AluOpType.mult)
                    nc.vector.tensor_tensor(t2[:], qx1_ps[:], sin_t, AluOpType.mult)
                    o1 = ropep.tile([128, SC], BF, tag="o1")
                    o2 = ropep.tile([128, SC], BF, tag="o2")
                    nc.vector.tensor_tensor(o1[:], p1[:], t1[:], AluOpType.subtract)
                    nc.vector.tensor_tensor(o2[:], p2[:], t2[:], AluOpType.add)
                    rope_r = [
                        ropep.tile([64, SC], BF, tag=f"rr{i}", name=f"rr{i}_{c}_{rep}")
                        for i in range(NHC)
                    ]
                    for h in range(NHC):
                        sl = slice(32 * h, 32 * h + 32)
                        nc.gpsimd.dma_start(rope_r[h][0:32, :], o1[sl, :])
                        nc.gpsimd.dma_start(rope_r[h][32:64, :], o2[sl, :])

                    # ---------------- phase B: attention per head ----------------
                    # head 0 runs its pre-diagonal k-tiles BEFORE the K/V
                    # up-projections, hiding the ckv AllGather + K-up latency.
                    nkt = 4 * c + 4

                    def attn_ktile(h, kt, den_ps, o_ps, rr):
                        diag = kt >= 4 * c
                        p = (kt - 4 * c) * 128 if diag else 0
                        s_ps = mmp.tile([128, SC], F32, tag="mm")
                        nc.tensor.matmul(
                            s_ps[:, p:SC],
                            k_nope[:, KT * kt : KT * (kt + 1)],
                            q_nope[h][:, p:SC],
                            start=True,
                            stop=False,
                        )
                        nc.tensor.matmul(
                            s_ps[:, p:SC],
                            k_rope[:, KT * kt : KT * (kt + 1)],
                            rr[:, p:SC],
                            start=False,
                            stop=True,
                        )
                        e = ep.tile([128, SC], BF, tag="e")
                        if diag:
                            tmp = ep.tile([128, 128], F32, tag="ediag", bufs=2,
                                          name=f"ediag_{c}_{h}_{kt}_{rep}")
                            nc.scalar.activation(
                                tmp[:], s_ps[:, p : p + 128], EXP, scale=SCALE
                            )
                            nc.vector.tensor_tensor(
                                e[:, p : p + 128], tmp[:], tri_t[:], AluOpType.mult
                            )
                            if p + 128 < SC:
                                nc.scalar.activation(
                                    e[:, p + 128 : SC], s_ps[:, p + 128 : SC],
                                    EXP, scale=SCALE,
                                )
                        else:
                            nc.scalar.activation(e[:], s_ps[:], EXP, scale=SCALE)
                        nc.tensor.matmul(
                            den_ps[:, p:SC],
                            ones_b[:],
                            e[:, p:SC],
                            start=(kt == 0),
                            stop=(kt == nkt - 1),
                        )
                        nc.tensor.matmul(
                            o_ps[:, p:SC],
                            v_t[kt][:],
                            e[:, p:SC],
                            start=(kt == 0),
                            stop=(kt == nkt - 1),
                        )

                    def head_tail(h, den_ps, o_ps):
                        recip = ropep.tile([128, SC], F32, tag="recip",
                                           name=f"recip_{c}_{h}_{rep}")
                        nc.vector.reciprocal(recip[:], den_ps[:])
                        on = onp.tile([128, SC], BF, tag=f"on{h}")
                        nc.vector.tensor_tensor(on[:], o_ps[:], recip[:],
                                                AluOpType.mult)
                        return on

                    o_norm = []
                    ckv_t = []
                    for l in range(4):
                        t = ckvp.tile([128, SC], BF, tag=f"ckv{l}")
                        eng = nc.sync if l % 2 == 0 else nc.scalar
                        eng.dma_start(t[:], ckvg[128 * l : 128 * (l + 1), :])
                        ckv_t.append(t)

                    # ---------------- K up-projection ----------------
                    ps_kn = mmp.tile([128, SC], F32, tag="mm")
                    for l in range(4):
                        nc.tensor.matmul(
                            ps_kn[:], wk_t[l][:, 0:128], ckv_t[l][:],
                            start=(l == 0), stop=(l == 3),
                        )
                    nc.vector.tensor_copy(k_nope[:, s0 : s0 + SC], ps_kn[:])

                    ps_kr = mmp.tile([64, SC], F32, tag="mm")
                    for l in range(4):
                        nc.tensor.matmul(
                            ps_kr[:], wk_t[l][:, 128:192], ckv_t[l][:],
                            start=(l == 0), stop=(l == 3),
                        )
                    kp = ropep.tile([64, SC], F32, tag="kp")
                    kt_ = ropep.tile([64, SC], F32, tag="kt_")
                    kts = ropep.tile([64, SC], F32, tag="kts")
                    nc.vector.tensor_tensor(
                        kp[:], ps_kr[:], cos_full[0:64, s0 : s0 + SC], AluOpType.mult
                    )
                    nc.vector.tensor_tensor(
                        kt_[:], ps_kr[:], sin_full[0:64, s0 : s0 + SC], AluOpType.mult
                    )
                    nc.gpsimd.dma_start(kts[0:32, :], kt_[32:64, :])
                    nc.gpsimd.dma_start(kts[32:64, :], kt_[0:32, :])
                    nc.vector.tensor_tensor(
                        k_rope[0:32, s0 : s0 + SC], kp[0:32, :], kts[0:32, :],
                        AluOpType.subtract,
                    )
                    nc.vector.tensor_tensor(
                        k_rope[32:64, s0 : s0 + SC], kp[32:64, :], kts[32:64, :],
                        AluOpType.add,
                    )

                    # ---------------- V up-projection ----------------
                    for ss in range(4):
                        ps_v = mmp.tile([128, D_V], F32, tag="mm")
                        for l in range(4):
                            nc.tensor.matmul(
                                ps_v[:],
                                ckv_t[l][:, 128 * ss : 128 * (ss + 1)],
                                wv_t[l][:],
                                start=(l == 0),
                                stop=(l == 3),
                            )
                        nc.vector.tensor_copy(v_t[4 * c + ss][:], ps_v[:])
                    for h in range(NHC):
                        den_ps = denp.tile([128, SC], F32, tag="den",
                                           name=f"den{h}_{c}_{rep}")
                        o_ps = op_.tile([128, SC], F32, tag="o",
                                        name=f"oo{h}_{c}_{rep}")
                        for kt in range(nkt):
                            attn_ktile(h, kt, den_ps, o_ps, rope_r[h])
                        o_norm.append(head_tail(h, den_ps, o_ps))

                    # ---------------- phase C: Y partial = O @ Wo -------------
                    for ss in range(4):
                        for np_ in range(2):
                            y_sb = yp_sb.tile([128, 1024], BF, tag="y",
                                              name=f"y_{c}_{ss}_{np_}_{rep}")
                            for nn in range(2):
                                n = 2 * np_ + nn
                                y_ps = mmp.tile([128, 512], F32, tag="mm",
                                                name=f"yps_{c}_{ss}_{n}_{rep}")
                                for h in range(NHC):
                                    nc.tensor.matmul(
                                        y_ps[:],
                                        o_norm[h][:, 128 * ss : 128 * (ss + 1)],
                                        wo_t[h][n][:],
                                        start=(h == 0),
                                        stop=(h == NHC - 1),
                                    )
                                cpy = nc.scalar.copy if nn == 0 else nc.vector.tensor_copy
                                cpy(y_sb[:, 512 * nn : 512 * (nn + 1)], y_ps[:])
                            nc.gpsimd.dma_start(
                                yp[s0 + 128 * ss : s0 + 128 * (ss + 1),
                                   1024 * np_ : 1024 * (np_ + 1)],
                                y_sb[:],
                            )

                pend_rs(NCHUNK - 1)

    nc.compile()
    return nc


def _host_inputs(hidden_states, Wqkv, Wk_up, Wv_up, Wo):
    """Build the 8 per-core input maps (each input byte uploaded once)."""
    lora_cols = np.ascontiguousarray(Wqkv[:, NH * D_QK :]).astype(BF16)  # [HID, LORA]

    per_g = {}
    for g in range(NKV):
        nopes, x1s, x2s = [], [], []
        for h in range(NHC):
            H = NHC * g + h
            base = H * D_QK
            nopes.append(Wqkv[:, base : base + D_NOPE])
            x1s.append(Wqkv[:, base + D_NOPE : base + D_NOPE + 32])
            x2s.append(Wqkv[:, base + D_NOPE + 32 : base + D_QK])
        w1q = np.concatenate(nopes + x1s + x2s, axis=1).astype(BF16)  # [HID, 768]
        wk = np.concatenate(
            [
                Wk_up[:, g * D_QK : g * D_QK + D_NOPE],
                Wk_up[:, g * D_QK + D_NOPE : g * D_QK + D_NOPE + 32],
                Wk_up[:, g * D_QK + D_NOPE + 32 : (g + 1) * D_QK],
            ],
            axis=1,
        ).astype(BF16)  # [LORA, 192]
        wv = np.ascontiguousarray(Wv_up[:, g * D_V : (g + 1) * D_V]).astype(BF16)
        wo = np.ascontiguousarray(Wo[g * NHC * D_V : (g + 1) * NHC * D_V, :]).astype(
            BF16
        )  # [512, HID]
        per_g[g] = (w1q, wk, wv, wo)

    in_maps = []
    for core in range(NCORES):
        b, g = core // NKV, core % NKV
        w1q, wk, wv, wo = per_g[g]
        hh = HID // 2
        wpk = np.concatenate(
            [
                np.ascontiguousarray(w1q[hh * b : hh * (b + 1), :]).ravel(),
                np.ascontiguousarray(
                    lora_cols[hh * b : hh * (b + 1), 128 * g : 128 * (g + 1)]
                ).ravel(),
                np.ascontiguousarray(wk[256 * b : 256 * (b + 1), :]).ravel(),
                np.ascontiguousarray(wv[256 * b : 256 * (b + 1), :]).ravel(),
                np.ascontiguousarray(wo[256 * b : 256 * (b + 1), :]).ravel(),
            ]
        )
        in_maps.append(
            {
                "xs": np.ascontiguousarray(
                    hidden_states[b, SC * g : SC * (g + 1), :]
                ).astype(BF16),
                "wpk": wpk,
            }
        )
    return in_maps


def _assemble(results):
    """Per-core [512, 2048] bf16 RS slices -> full [B, S, HID] fp32."""
    out = np.zeros((B, S, HID), dtype=np.float32)
    for core in range(NCORES):
        b, g = core // NKV, core % NKV
        y = np.asarray(results[core]["y"]).astype(np.float32)
        for c in range(NCHUNK):
            r0 = SC * c + 128 * g
            out[b, r0 : r0 + 128, :] = y[128 * c : 128 * (c + 1), :]
    return out


def _get_runner(nc):
    """Cached jitted dispatch (axon/PJRT path only) — avoids the per-call
    jit retrace that run_bass_kernel_spmd's redirect pays."""
    if "runner" in _PROGRAM_CACHE:
        return _PROGRAM_CACHE["runner"]
    import jax
    from jax.experimental.shard_map import shard_map
    from jax.sharding import Mesh, NamedSharding, PartitionSpec

    from concourse.bass2jax import (
        _bass_exec_p,
        install_neuronx_cc_hook,
        partition_id_tensor,
    )

    install_neuronx_cc_hook()
    partition_name = nc.partition_id_tensor.name if nc.partition_id_tensor else None
    in_names, out_names, out_avals = [], [], []
    for alloc in nc.m.functions[0].allocations:
        if not isinstance(alloc, mybir.MemoryLocationSet):
            continue
        name = alloc.memorylocations[0].name
        if alloc.kind == "ExternalInput":
            if name != partition_name:
                in_names.append(name)
        elif alloc.kind == "ExternalOutput":
            out_names.append(name)
            out_avals.append(
                jax.core.ShapedArray(
                    tuple(alloc.tensor_shape), mybir.dt.np(alloc.dtype)
                )
            )
    all_names = in_names + out_names + ([partition_name] if partition_name else [])

    def _body(*args):
        operands = list(args)
        if partition_name is not None:
            operands.append(partition_id_tensor())
        return tuple(
            _bass_exec_p.bind(
                *operands,
                out_avals=tuple(out_avals),
                in_names=tuple(all_names),
                out_names=tuple(out_names),
                lowering_input_output_aliases=(),
                sim_require_finite=True,
                sim_require_nnan=True,
                nc=nc,
            )
        )

    devices = jax.devices()[:NCORES]
    mesh = Mesh(np.asarray(devices), ("core",))
    sharded = jax.jit(
        shard_map(
            _body,
            mesh=mesh,
            in_specs=(PartitionSpec("core"),) * (len(in_names) + len(out_names)),
            out_specs=(PartitionSpec("core"),) * len(out_names),
            check_rep=False,
        ),
        keep_unused=True,
    )
    sh = NamedSharding(mesh, PartitionSpec("core"))
    dev_zeros = [
        jax.device_put(np.zeros((NCORES * a.shape[0], *a.shape[1:]), a.dtype), sh)
        for a in out_avals
    ]
    runner = (sharded, in_names, out_names, dev_zeros)
    _PROGRAM_CACHE["runner"] = runner
    return runner


def kernel(hidden_states, Wqkv, Wk_up, Wv_up, Wo):
    hidden_states = np.asarray(hidden_states, dtype=np.float32)
    Wqkv = np.asarray(Wqkv, dtype=np.float32)
    Wk_up = np.asarray(Wk_up, dtype=np.float32)
    Wv_up = np.asarray(Wv_up, dtype=np.float32)
    Wo = np.asarray(Wo, dtype=np.float32)

    if "nc" not in _PROGRAM_CACHE:
        _PROGRAM_CACHE["nc"] = _build_program()
    nc = _PROGRAM_CACHE["nc"]

    in_maps = _host_inputs(hidden_states, Wqkv, Wk_up, Wv_up, Wo)

    try:
        from concourse._compat import axon_active

        use_runner = axon_active()
    except Exception:
        use_runner = False

    if use_runner:
        sharded, in_names, out_names, dev_zeros = _get_runner(nc)
        concat_in = [
            np.concatenate([np.asarray(in_maps[c][nm]) for c in range(NCORES)], axis=0)
            for nm in in_names
        ]
        outs = sharded(*concat_in, *dev_zeros)
        y_all = np.asarray(outs[out_names.index("y")]).reshape(NCORES, SC, HID)
        results = [{"y": y_all[c]} for c in range(NCORES)]
    else:
        results = run_bass_kernel_spmd(nc, in_maps, list(range(NCORES))).results
    return _assemble(results)


if __name__ == "__main__":
    rng = np.random.default_rng(0)
    hs = rng.standard_normal((B, S, HID)).astype(np.float32)
    wqkv = rng.standard_normal((HID, NH * D_QK + LORA)).astype(np.float32) * 0.02
    wk = rng.standard_normal((LORA, NKV * D_QK)).astype(np.float32) * 0.04
    wv = rng.standard_normal((LORA, NKV * D_V)).astype(np.float32) * 0.04
    wo = rng.standard_normal((NH * D_V, HID)).astype(np.float32) * 0.02
    y = kernel(hs, wqkv, wk, wv, wo)
    print("kernel output", y.shape, y.dtype, float(np.abs(y).max()))


# revision 51
# speedup vs baseline: 1.1390x; 1.1390x over previous
"""Fused MLA-with-GQA attention kernel for 8 Trainium2 NeuronCores.

Sharding: 8 cores = 2 (batch) x 4 (kv-head groups); core = 4*b + g owns
batch b, query heads 4g..4g+3 and kv head g.

Host->device traffic is minimized: every input byte is uploaded exactly
once (bf16), and on-device AllGathers over the chip interconnect rebuild
the per-core operands:
  xs   [512,2048]  X rows owned by this core      -> AllGather {b-group}
  w1q  [1024,768]  half of group-g Wqkv q-cols    -> AllGather {pair}
  lra  [256,512]   1/8 of the shared LoRA cols    -> AllGather {all 8}
  wk/wv [256,*]    half of group-g up-projections -> AllGather {pair}
  wo   [256,2048]  quarter... half of g-rows of Wo-> AllGather {pair}
Rope cos/sin tables, the triangular mask and the transpose identity are
compile-time constants embedded in the NEFF. The partial outputs are
ReduceScattered per s-chunk across each batch group, so every core
downloads only its own [512, 2048] bf16 slice of Y.

On-device layout is fully transposed (feature-major); X^T is produced
on-device with PE transposes of the gathered X. All matmul operands are
bf16 (full-rate), accumulation in fp32 PSUM. Causal structure: k-tiles
above the diagonal are skipped; diagonal k-tiles compute the column
sub-range [p:512] only, with a triangular mask multiply after exp.
"""

import math
import sys

import numpy as np

for _p in ("/opt/trn_rl_repo", "/root/.axon_site/_ro/trn_rl_repo"):
    if _p not in sys.path:
        try:
            import os

            if os.path.isdir(_p):
                sys.path.insert(0, _p)
        except Exception:
            pass

import ml_dtypes

import concourse.bacc as bacc
import concourse.mybir as mybir
import concourse.tile as tile
from concourse.alu_op_type import AluOpType
from concourse.bass_utils import run_bass_kernel_spmd

BF16 = ml_dtypes.bfloat16

# ---- problem constants (hardcoded; kernel.py must be self-contained) ----
HID = 2048
NH = 16
NKV = 4
NG = NH // NKV  # 4 q heads per kv head
LORA = 512
D_ROPE = 64
D_NOPE = 128
D_V = 128
D_QK = D_NOPE + D_ROPE  # 192
B, S = 2, 2048
ROPE_BASE = 10000.0
NCORES = 8

NHC = NG  # heads per core = 4
SC = 512  # s-chunk width
NCHUNK = S // SC  # 4
KT = 128  # k tile
NKT_TOT = S // KT  # 16
SCALE = 1.0 / math.sqrt(D_QK)
QCOLS = NHC * D_QK  # 768 packed q columns per group

# packed per-core weight buffer (bf16 elements): halves of w1q, lra, wk,
# wv, wo concatenated flat; pair-AllGather rebuilds both halves.
OFF_W1Q = 0
OFF_LRA = OFF_W1Q + (HID // 2) * QCOLS      # 786432
OFF_WK = OFF_LRA + (HID // 2) * (LORA // 4)  # 917504
OFF_WV = OFF_WK + (LORA // 2) * D_QK         # 966656
OFF_WO = OFF_WV + (LORA // 2) * D_V          # 999424
WFLAT = OFF_WO + (NHC * D_V // 2) * HID      # 1523712

G_BATCH = [[0, 1, 2, 3], [4, 5, 6, 7]]
G_PAIR = [[0, 4], [1, 5], [2, 6], [3, 7]]
G_ALL = [[0, 1, 2, 3, 4, 5, 6, 7]]

F32 = mybir.dt.float32
BF = mybir.dt.bfloat16
EXP = mybir.ActivationFunctionType.Exp

_PROGRAM_CACHE = {}


def _rope_tables():
    inv_freq = 1.0 / (ROPE_BASE ** (np.arange(0, D_ROPE, 2, dtype=np.float64) / D_ROPE))
    t = np.arange(S, dtype=np.float64)
    freqs = np.outer(t, inv_freq)  # [S, 32]
    return np.cos(freqs).T, np.sin(freqs).T  # each [32, S]


def _build_program(reps: int = 1, single: bool = False):
    """single=True replaces collectives with local DMAs (for CoreSim timing
    only; numerics are wrong in that mode)."""
    nc = bacc.Bacc("TRN2", target_bir_lowering=False, debug=False, num_devices=NCORES)

    # ---- per-core uploads (bf16) ----
    xs_d = nc.dram_tensor("xs", [SC, HID], BF, kind="ExternalInput").ap()
    wpk_d = nc.dram_tensor("wpk", [WFLAT], BF, kind="ExternalInput").ap()
    y_d = nc.dram_tensor("y", [SC, HID], BF, kind="ExternalOutput").ap()

    # ---- compile-time constants embedded in the NEFF ----
    cosT, sinT = _rope_tables()
    cs_d = nc.inline_tensor(
        np.concatenate([cosT, sinT], axis=0).astype(BF16), name="cs"
    ).ap()  # [64, S]
    tri_d = nc.inline_tensor(
        np.triu(np.ones((128, 128))).astype(BF16), name="trimask"
    ).ap()
    eye_d = nc.inline_tensor(np.eye(128).astype(BF16), name="eye128").ap()

    # ---- internal DRAM: collective bounces and gathered operands ----
    wpk_b = nc.dram_tensor("wpk_b", [WFLAT], BF, kind="Internal").ap()
    wpkg = nc.dram_tensor("wpkg", [2 * WFLAT], BF, kind="Internal").ap()
    ckv_b = nc.dram_tensor("ckv_b", [LORA // 4, SC], BF, kind="Internal").ap()
    xt_b = nc.dram_tensor("xt_b", [HID, SC], BF, kind="Internal").ap()
    xtg = nc.dram_tensor("xtg", [NCHUNK * HID, SC], BF, kind="Internal").ap()
    ckvg = nc.dram_tensor("ckvg", [LORA, SC], BF, kind="Internal").ap()

    def wview(b, off, lr, rows, cols):
        base = b * WFLAT + off + lr * cols
        return wpkg[base : base + rows * cols].rearrange(
            "(r c) -> r c", r=rows, c=cols
        )
    yp = nc.dram_tensor("yp", [S, HID], BF, kind="Internal").ap()
    yrs = nc.dram_tensor("yrs", [SC, HID], BF, kind="Internal").ap()

    from contextlib import ExitStack

    with tile.TileContext(nc) as tc:
        with ExitStack() as ctx:
            constp = ctx.enter_context(tc.tile_pool(name="const", bufs=1))
            stagep = ctx.enter_context(tc.tile_pool(name="stage", bufs=1))
            wop = ctx.enter_context(tc.tile_pool(name="wo", bufs=1))
            w1p = ctx.enter_context(tc.tile_pool(name="w1s", bufs=1))
            xrp = ctx.enter_context(tc.tile_pool(name="xr", bufs=1))
            xp = ctx.enter_context(tc.tile_pool(name="x", bufs=1))
            qnp = ctx.enter_context(tc.tile_pool(name="qn", bufs=1))
            ckvp = ctx.enter_context(tc.tile_pool(name="ckv", bufs=1))
            kfp = ctx.enter_context(tc.tile_pool(name="kf", bufs=1))
            vp = ctx.enter_context(tc.tile_pool(name="v", bufs=1))
            ropep = ctx.enter_context(tc.tile_pool(name="rope", bufs=1))
            ep = ctx.enter_context(tc.tile_pool(name="e", bufs=3))
            onp = ctx.enter_context(tc.tile_pool(name="on", bufs=1))
            yp_sb = ctx.enter_context(tc.tile_pool(name="y", bufs=2))
            mmp = ctx.enter_context(tc.tile_pool(name="mm", bufs=6, space="PSUM"))
            denp = ctx.enter_context(tc.tile_pool(name="den", bufs=1, space="PSUM"))
            op_ = ctx.enter_context(tc.tile_pool(name="o", bufs=1, space="PSUM"))

            # ---------------- constants ----------------
            tri_t = constp.tile([128, 128], BF, tag="tri")
            nc.scalar.dma_start(tri_t[:], tri_d[:])
            eye_t = constp.tile([128, 128], BF, tag="eye")
            nc.sync.dma_start(eye_t[:], eye_d[:])

            ones_f = constp.tile([128, 128], F32, tag="ones_f")
            nc.gpsimd.memset(ones_f[:], 1.0)
            ones_b = constp.tile([128, 128], BF, tag="ones_b")
            nc.scalar.copy(ones_b[:], ones_f[:])

            # full-S rope tables, 4 head-copies of 32 rows each
            cos_full = constp.tile([128, S], BF, tag="cosf")
            sin_full = constp.tile([128, S], BF, tag="sinf")
            for h in range(4):
                nc.scalar.dma_start(cos_full[32 * h : 32 * h + 32, :], cs_d[0:32, :])
                nc.sync.dma_start(sin_full[32 * h : 32 * h + 32, :], cs_d[32:64, :])

            def stage(bounce, src, rows, cols, tag):
                """ExternalInput -> SBUF -> internal DRAM bounce."""
                for i, r0 in enumerate(range(0, rows, 128)):
                    r1 = min(r0 + 128, rows)
                    t = stagep.tile([128, cols], BF, tag=tag, name=f"{tag}_{r0}")
                    reng = nc.sync if i % 2 == 0 else nc.scalar
                    reng.dma_start(t[0 : r1 - r0, :], src[r0:r1, :])
                    nc.gpsimd.dma_start(bounce[r0:r1, :], t[0 : r1 - r0, :])

            def gather(kind_groups, in_ap, out_ap, nsplit):
                if single:
                    # timing-only: copy own contribution to slot 0
                    rows = in_ap.shape[0]
                    for r0 in range(0, rows, 128):
                        r1 = min(r0 + 128, rows)
                        t = stagep.tile(
                            [128, in_ap.shape[1]], BF,
                            tag=f"agt_{out_ap.tensor.name}",
                            name=f"agt_{out_ap.tensor.name}_{r0}",
                        )
                        nc.sync.dma_start(t[0 : r1 - r0, :], in_ap[r0:r1, :])
                        nc.gpsimd.dma_start(out_ap[r0:r1, :], t[0 : r1 - r0, :])
                else:
                    nc.gpsimd.collective_compute(
                        "AllGather",
                        mybir.AluOpType.bypass,
                        replica_groups=kind_groups,
                        ins=[in_ap[:]],
                        outs=[out_ap[:]],
                    )

            # persistent K / V state across chunks
            k_nope = kfp.tile([128, S], BF, tag="k_nope")
            k_rope = kfp.tile([64, S], BF, tag="k_rope")
            v_t = [
                vp.tile([128, D_V], BF, tag=f"v{i}", name=f"v{i}")
                for i in range(NKT_TOT)
            ]

            for rep in range(reps):
                # ---- transpose own X slice, stage bounces, then gather ----
                # The own-slice transposes run before any collective; the
                # gathered tensor is already feature-major per source chunk.
                xo_t = [
                    xp.tile([128, SC], BF, tag=f"xo{ht}", name=f"xo{ht}_{rep}")
                    for ht in range(16)
                ]
                for k0 in (0, 2):
                    xsrs = []
                    for dk in range(2):
                        k = k0 + dk
                        xsr = xrp.tile([128, HID], BF, tag="xrow", bufs=2,
                                       name=f"xsr_{k}_{rep}")
                        nc.sync.dma_start(xsr[:], xs_d[128 * k : 128 * (k + 1), :])
                        xsrs.append(xsr)
                    for ht in range(16):
                        # two 128-col transposes share one PSUM tile so a
                        # single DVE copy moves both k-blocks
                        ps_t = mmp.tile([128, 256], BF, tag="mm")
                        for dk in range(2):
                            nc.tensor.transpose(
                                ps_t[:, 128 * dk : 128 * (dk + 1)],
                                xsrs[dk][:, 128 * ht : 128 * (ht + 1)],
                                eye_t[:],
                            )
                        nc.vector.tensor_copy(
                            xo_t[ht][:, 128 * k0 : 128 * (k0 + 2)], ps_t[:]
                        )
                        if k0 == 2:
                            # tile ht is complete - ship it while later ht
                            # transposes are still running
                            weng = nc.gpsimd if ht % 2 == 0 else nc.scalar
                            weng.dma_start(
                                xt_b[128 * ht : 128 * (ht + 1), :], xo_t[ht][:]
                            )

                # gathers are a serial stream: X first (it gates the first
                # compute), then the single packed-weights pair-gather.
                PCOL = 1984  # WFLAT = 128 * 11904 = 128 * 6 * 1984
                for i in range(6):
                    o0 = i * 128 * PCOL
                    t = stagep.tile([128, PCOL], BF, tag="sg_wpk",
                                    name=f"sg_wpk_{i}_{rep}")
                    reng = nc.sync if i % 2 == 0 else nc.scalar
                    reng.dma_start(
                        t[:],
                        wpk_d[o0 : o0 + 128 * PCOL].rearrange(
                            "(r c) -> r c", r=128, c=PCOL
                        ),
                    )
                    nc.gpsimd.dma_start(
                        wpk_b[o0 : o0 + 128 * PCOL].rearrange(
                            "(r c) -> r c", r=128, c=PCOL
                        ),
                        t[:],
                    )
                gather(G_BATCH, xt_b, xtg, 4)
                if single:
                    for i in range(6):
                        o0 = i * 128 * PCOL
                        t = stagep.tile([128, PCOL], BF, tag="sg_wpk",
                                        name=f"agt_wpk_{i}_{rep}")
                        nc.sync.dma_start(
                            t[:],
                            wpk_b[o0 : o0 + 128 * PCOL].rearrange(
                                "(r c) -> r c", r=128, c=PCOL
                            ),
                        )
                        nc.gpsimd.dma_start(
                            wpkg[o0 : o0 + 128 * PCOL].rearrange(
                                "(r c) -> r c", r=128, c=PCOL
                            ),
                            t[:],
                        )
                else:
                    nc.gpsimd.collective_compute(
                        "AllGather",
                        mybir.AluOpType.bypass,
                        replica_groups=G_PAIR,
                        ins=[wpk_b[:]],
                        outs=[wpkg[:]],
                    )

                # wk/wv l-tiles
                wk_t = []
                wv_t = []
                for l in range(4):
                    b_, lr = l // 2, 128 * l - 256 * (l // 2)
                    t = constp.tile([128, D_QK], BF, tag=f"wk{l}",
                                    name=f"wk{l}_{rep}")
                    nc.gpsimd.dma_start(t[:], wview(b_, OFF_WK, lr, 128, D_QK))
                    wk_t.append(t)
                    t = constp.tile([128, D_V], BF, tag=f"wv{l}",
                                    name=f"wv{l}_{rep}")
                    nc.gpsimd.dma_start(t[:], wview(b_, OFF_WV, lr, 128, D_V))
                    wv_t.append(t)

                # wo resident: per (head, n-block) tiles [128, 512]
                wo_t = [[None] * 4 for _ in range(NHC)]
                for h in range(NHC):
                    for n in range(4):
                        t = wop.tile([128, 512], BF, tag=f"wo{h}_{n}",
                                     name=f"wo{h}_{n}_{rep}")
                        b_, lr = h // 2, 128 * h - 256 * (h // 2)
                        nc.gpsimd.dma_start(
                            t[:],
                            wview(b_, OFF_WO, lr, 128, HID)[
                                :, 512 * n : 512 * (n + 1)
                            ],
                        )
                        wo_t[h][n] = t

                # ---- resident W1 (q-cols + own LoRA block), loaded once ----
                w1_t = {}
                wl_t = {}
                # wl (LoRA) tiles first: j=6 is the first matmul group and
                # needs only these 0.5MB, not the 3MB of w1 q-columns
                for ht in range(16):
                    b_, lr = ht // 8, 128 * ht - 1024 * (ht // 8)
                    t2 = w1p.tile([128, 128], BF, tag=f"wl_{ht}",
                                  name=f"wl_{ht}_{rep}")
                    eng = nc.scalar if ht % 2 == 0 else nc.sync
                    eng.dma_start(t2[:], wview(b_, OFF_LRA, lr, 128, LORA // 4))
                    wl_t[ht] = t2
                for ht in range(16):
                    b_, lr = ht // 8, 128 * ht - 1024 * (ht // 8)
                    t = w1p.tile([128, QCOLS], BF, tag=f"w1_{ht}",
                                 name=f"w1_{ht}_{rep}")
                    eng = nc.scalar if ht % 2 == 0 else nc.sync
                    eng.dma_start(t[:], wview(b_, OFF_W1Q, lr, 128, QCOLS))
                    w1_t[ht] = t

                def pend_rs(c):
                    s0 = SC * c
                    if single:
                        t = stagep.tile([128, HID], BF, tag="rst",
                                        name=f"rst_{c}_{rep}")
                        nc.sync.dma_start(t[:], yp[s0 : s0 + 128, :])
                        nc.gpsimd.dma_start(yrs[128 * c : 128 * (c + 1), :], t[:])
                    else:
                        nc.gpsimd.collective_compute(
                            "ReduceScatter",
                            mybir.AluOpType.add,
                            replica_groups=G_BATCH,
                            ins=[yp[s0 : s0 + SC, :]],
                            outs=[yrs[128 * c : 128 * (c + 1), :]],
                        )
                    # copy the scattered piece to the output tensor
                    t = stagep.tile([128, HID], BF, tag="sgbig",
                                    name=f"yout_{c}_{rep}")
                    nc.sync.dma_start(t[:], yrs[128 * c : 128 * (c + 1), :])
                    nc.gpsimd.dma_start(y_d[128 * c : 128 * (c + 1), :], t[:])

                for c in range(NCHUNK):
                    s0 = SC * c

                    # x_t tiles for this chunk straight from the gathered X^T
                    x_t = []
                    for ht in range(16):
                        t = xp.tile([128, SC], BF, tag=f"x{ht}", bufs=2,
                                    name=f"x{ht}_{c}_{rep}")
                        eng = nc.sync if ht % 2 == 0 else nc.scalar
                        eng.dma_start(
                            t[:],
                            xtg[HID * c + 128 * ht : HID * c + 128 * (ht + 1), :],
                        )
                        x_t.append(t)

                    # ------- phase A: C1 = X @ W1 (transposed) -------
                    # q columns (6 j's) + this core's 128 LoRA features (1 j);
                    # the other 3 LoRA blocks come from the batch-group
                    # AllGather of ckv below.
                    q_nope = []
                    qx1_ps = qx2_ps = None
                    for j in (6, 0, 1, 2, 3, 4, 5):
                        ps = mmp.tile([128, SC], F32, tag="mm")
                        for ht in range(16):
                            lhsT = (
                                w1_t[ht][:, 128 * j : 128 * (j + 1)]
                                if j < 6
                                else wl_t[ht][:]
                            )
                            nc.tensor.matmul(
                                ps[:],
                                lhsT,
                                x_t[ht][:],
                                start=(ht == 0),
                                stop=(ht == 15),
                            )
                        if j < 4:
                            t = qnp.tile([128, SC], BF, tag=f"qn{j}")
                            nc.scalar.copy(t[:], ps[:])
                            q_nope.append(t)
                        elif j == 4:
                            qx1_ps = ps
                        elif j == 5:
                            qx2_ps = ps
                        else:
                            ckv_own = ckvp.tile([128, SC], BF, tag="ckv_own")
                            nc.scalar.copy(ckv_own[:], ps[:])
                            nc.gpsimd.dma_start(ckv_b[:], ckv_own[:])
                            # gather the 4 LoRA blocks of c_kv across the
                            # batch group; hidden under the remaining q j's
                            if single:
                                t = stagep.tile([128, SC], BF, tag="agt_ckvg",
                                                name=f"agt_ckvg_{c}_{rep}")
                                nc.sync.dma_start(t[:], ckv_b[:])
                                nc.gpsimd.dma_start(ckvg[0:128, :], t[:])
                            else:
                                nc.gpsimd.collective_compute(
                                    "AllGather",
                                    mybir.AluOpType.bypass,
                                    replica_groups=G_BATCH,
                                    ins=[ckv_b[:]],
                                    outs=[ckvg[:]],
                                )
                            # deferred ReduceScatter of the previous chunk's
                            # Y (keeps the ckv AllGather ahead in the stream)
                            if c > 0:
                                pend_rs(c - 1)
                    cos_t = cos_full[:, s0 : s0 + SC]
                    sin_t = sin_full[:, s0 : s0 + SC]

                    # ---- Q rope (4 heads batched in 128 partitions) ----
                    p1 = ropep.tile([128, SC], F32, tag="p1")
                    t1 = ropep.tile([128, SC], F32, tag="t1")
                    p2 = ropep.tile([128, SC], F32, tag="p2")
                    t2 = ropep.tile([128, SC], F32, tag="t2")
                    nc.vector.tensor_tensor(p1[:], qx1_ps[:], cos_t, AluOpType.mult)
                    nc.vector.tensor_tensor(t1[:], qx2_ps[:], sin_t, AluOpType.mult)
                    nc.vector.tensor_tensor(p2[:], qx2_ps[:], cos_t, AluOpType.mult)
                    nc.vector.tensor_tensor(t2[:], qx1_ps[:], sin_t, AluOpType.mult)
                    o1 = ropep.tile([128, SC], BF, tag="o1")
                    o2 = ropep.tile([128, SC], BF, tag="o2")
                    nc.vector.tensor_tensor(o1[:], p1[:], t1[:], AluOpType.subtract)
                    nc.vector.tensor_tensor(o2[:], p2[:], t2[:], AluOpType.add)
                    rope_r = [
                        ropep.tile([64, SC], BF, tag=f"rr{i}", name=f"rr{i}_{c}_{rep}")
                        for i in range(NHC)
                    ]
                    for h in range(NHC):
                        sl = slice(32 * h, 32 * h + 32)
                        nc.gpsimd.dma_start(rope_r[h][0:32, :], o1[sl, :])
                        nc.gpsimd.dma_start(rope_r[h][32:64, :], o2[sl, :])

                    # ---------------- phase B: attention per head ----------------
                    # head 0 runs its pre-diagonal k-tiles BEFORE the K/V
                    # up-projections, hiding the ckv AllGather + K-up latency.
                    nkt = 4 * c + 4

                    def attn_ktile(h, kt, den_ps, o_ps, rr):
                        diag = kt >= 4 * c
                        p = (kt - 4 * c) * 128 if diag else 0
                        s_ps = mmp.tile([128, SC], F32, tag="mm")
                        nc.tensor.matmul(
                            s_ps[:, p:SC],
                            k_nope[:, KT * kt : KT * (kt + 1)],
                            q_nope[h][:, p:SC],
                            start=True,
                            stop=False,
                        )
                        nc.tensor.matmul(
                            s_ps[:, p:SC],
                            k_rope[:, KT * kt : KT * (kt + 1)],
                            rr[:, p:SC],
                            start=False,
                            stop=True,
                        )
                        e = ep.tile([128, SC], BF, tag="e")
                        if diag:
                            tmp = ep.tile([128, 128], F32, tag="ediag", bufs=2,
                                          name=f"ediag_{c}_{h}_{kt}_{rep}")
                            nc.scalar.activation(
                                tmp[:], s_ps[:, p : p + 128], EXP, scale=SCALE
                            )
                            nc.vector.tensor_tensor(
                                e[:, p : p + 128], tmp[:], tri_t[:], AluOpType.mult
                            )
                            if p + 128 < SC:
                                nc.scalar.activation(
                                    e[:, p + 128 : SC], s_ps[:, p + 128 : SC],
                                    EXP, scale=SCALE,
                                )
                        else:
                            nc.scalar.activation(e[:], s_ps[:], EXP, scale=SCALE)
                        nc.tensor.matmul(
                            den_ps[:, p:SC],
                            ones_b[:],
                            e[:, p:SC],
                            start=(kt == 0),
                            stop=(kt == nkt - 1),
                        )
                        nc.tensor.matmul(
                            o_ps[:, p:SC],
                            v_t[kt][:],
                            e[:, p:SC],
                            start=(kt == 0),
                            stop=(kt == nkt - 1),
                        )

                    def head_tail(h, den_ps, o_ps):
                        recip = ropep.tile([128, SC], F32, tag="recip",
                                           name=f"recip_{c}_{h}_{rep}")
                        nc.vector.reciprocal(recip[:], den_ps[:])
                        on = onp.tile([128, SC], BF, tag=f"on{h}")
                        nc.vector.tensor_tensor(on[:], o_ps[:], recip[:],
                                                AluOpType.mult)
                        return on

                    o_norm = []
                    ckv_t = []
                    for l in range(4):
                        t = ckvp.tile([128, SC], BF, tag=f"ckv{l}")
                        eng = nc.sync if l % 2 == 0 else nc.scalar
                        eng.dma_start(t[:], ckvg[128 * l : 128 * (l + 1), :])
                        ckv_t.append(t)

                    # ---------------- K up-projection ----------------
                    ps_kn = mmp.tile([128, SC], F32, tag="mm")
                    for l in range(4):
                        nc.tensor.matmul(
                            ps_kn[:], wk_t[l][:, 0:128], ckv_t[l][:],
                            start=(l == 0), stop=(l == 3),
                        )
                    nc.vector.tensor_copy(k_nope[:, s0 : s0 + SC], ps_kn[:])

                    ps_kr = mmp.tile([64, SC], F32, tag="mm")
                    for l in range(4):
                        nc.tensor.matmul(
                            ps_kr[:], wk_t[l][:, 128:192], ckv_t[l][:],
                            start=(l == 0), stop=(l == 3),
                        )
                    kp = ropep.tile([64, SC], F32, tag="kp")
                    kt_ = ropep.tile([64, SC], F32, tag="kt_")
                    kts = ropep.tile([64, SC], F32, tag="kts")
                    nc.vector.tensor_tensor(
                        kp[:], ps_kr[:], cos_full[0:64, s0 : s0 + SC], AluOpType.mult
                    )
                    nc.vector.tensor_tensor(
                        kt_[:], ps_kr[:], sin_full[0:64, s0 : s0 + SC], AluOpType.mult
                    )
                    nc.gpsimd.dma_start(kts[0:32, :], kt_[32:64, :])
                    nc.gpsimd.dma_start(kts[32:64, :], kt_[0:32, :])
                    nc.vector.tensor_tensor(
                        k_rope[0:32, s0 : s0 + SC], kp[0:32, :], kts[0:32, :],
                        AluOpType.subtract,
                    )
                    nc.vector.tensor_tensor(
                        k_rope[32:64, s0 : s0 + SC], kp[32:64, :], kts[32:64, :],
                        AluOpType.add,
                    )

                    # ---------------- V up-projection ----------------
                    for ss in range(4):
                        ps_v = mmp.tile([128, D_V], F32, tag="mm")
                        for l in range(4):
                            nc.tensor.matmul(
                                ps_v[:],
                                ckv_t[l][:, 128 * ss : 128 * (ss + 1)],
                                wv_t[l][:],
                                start=(l == 0),
                                stop=(l == 3),
                            )
                        nc.vector.tensor_copy(v_t[4 * c + ss][:], ps_v[:])
                    for h in range(NHC):
                        den_ps = denp.tile([128, SC], F32, tag="den",
                                           name=f"den{h}_{c}_{rep}")
                        o_ps = op_.tile([128, SC], F32, tag="o",
                                        name=f"oo{h}_{c}_{rep}")
                        for kt in range(nkt):
                            attn_ktile(h, kt, den_ps, o_ps, rope_r[h])
                        o_norm.append(head_tail(h, den_ps, o_ps))

                    # ---------------- phase C: Y partial = O @ Wo -------------
                    for ss in range(4):
                        for np_ in range(2):
                            y_sb = yp_sb.tile([128, 1024], BF, tag="y",
                                              name=f"y_{c}_{ss}_{np_}_{rep}")
                            for nn in range(2):
                                n = 2 * np_ + nn
                                y_ps = mmp.tile([128, 512], F32, tag="mm",
                                                name=f"yps_{c}_{ss}_{n}_{rep}")
                                for h in range(NHC):
                                    nc.tensor.matmul(
                                        y_ps[:],
                                        o_norm[h][:, 128 * ss : 128 * (ss + 1)],
                                        wo_t[h][n][:],
                                        start=(h == 0),
                                        stop=(h == NHC - 1),
                                    )
                                cpy = nc.scalar.copy if nn == 0 else nc.vector.tensor_copy
                                cpy(y_sb[:, 512 * nn : 512 * (nn + 1)], y_ps[:])
                            nc.gpsimd.dma_start(
                                yp[s0 + 128 * ss : s0 + 128 * (ss + 1),
                                   1024 * np_ : 1024 * (np_ + 1)],
                                y_sb[:],
                            )

                pend_rs(NCHUNK - 1)

    nc.compile()
    return nc


def _host_inputs(hidden_states, Wqkv, Wk_up, Wv_up, Wo):
    """Build the 8 per-core input maps (each input byte uploaded once)."""
    lora_cols = np.ascontiguousarray(Wqkv[:, NH * D_QK :]).astype(BF16)  # [HID, LORA]

    per_g = {}
    for g in range(NKV):
        nopes, x1s, x2s = [], [], []
        for h in range(NHC):
            H = NHC * g + h
            base = H * D_QK
            nopes.append(Wqkv[:, base : base + D_NOPE])
            x1s.append(Wqkv[:, base + D_NOPE : base + D_NOPE + 32])
            x2s.append(Wqkv[:, base + D_NOPE + 32 : base + D_QK])
        w1q = np.concatenate(nopes + x1s + x2s, axis=1).astype(BF16)  # [HID, 768]
        wk = np.concatenate(
            [
                Wk_up[:, g * D_QK : g * D_QK + D_NOPE],
                Wk_up[:, g * D_QK + D_NOPE : g * D_QK + D_NOPE + 32],
                Wk_up[:, g * D_QK + D_NOPE + 32 : (g + 1) * D_QK],
            ],
            axis=1,
        ).astype(BF16)  # [LORA, 192]
        wv = np.ascontiguousarray(Wv_up[:, g * D_V : (g + 1) * D_V]).astype(BF16)
        wo = np.ascontiguousarray(Wo[g * NHC * D_V : (g + 1) * NHC * D_V, :]).astype(
            BF16
        )  # [512, HID]
        per_g[g] = (w1q, wk, wv, wo)

    in_maps = []
    for core in range(NCORES):
        b, g = core // NKV, core % NKV
        w1q, wk, wv, wo = per_g[g]
        hh = HID // 2
        wpk = np.concatenate(
            [
                np.ascontiguousarray(w1q[hh * b : hh * (b + 1), :]).ravel(),
                np.ascontiguousarray(
                    lora_cols[hh * b : hh * (b + 1), 128 * g : 128 * (g + 1)]
                ).ravel(),
                np.ascontiguousarray(wk[256 * b : 256 * (b + 1), :]).ravel(),
                np.ascontiguousarray(wv[256 * b : 256 * (b + 1), :]).ravel(),
                np.ascontiguousarray(wo[256 * b : 256 * (b + 1), :]).ravel(),
            ]
        )
        in_maps.append(
            {
                "xs": np.ascontiguousarray(
                    hidden_states[b, SC * g : SC * (g + 1), :]
                ).astype(BF16),
                "wpk": wpk,
            }
        )
    return in_maps


def _assemble(results):
    """Per-core [512, 2048] bf16 RS slices -> full [B, S, HID] fp32."""
    out = np.zeros((B, S, HID), dtype=np.float32)
    for core in range(NCORES):
        b, g = core // NKV, core % NKV
        y = np.asarray(results[core]["y"]).astype(np.float32)
        for c in range(NCHUNK):
            r0 = SC * c + 128 * g
            out[b, r0 : r0 + 128, :] = y[128 * c : 128 * (c + 1), :]
    return out


def _get_runner(nc):
    """Cached jitted dispatch (axon/PJRT path only) — avoids the per-call
    jit retrace that run_bass_kernel_spmd's redirect pays."""
    if "runner" in _PROGRAM_CACHE:
        return _PROGRAM_CACHE["runner"]
    import jax
    from jax.experimental.shard_map import shard_map
    from jax.sharding import Mesh, NamedSharding, PartitionSpec

    from concourse.bass2jax import (
        _bass_exec_p,
        install_neuronx_cc_hook,
        partition_id_tensor,
    )

    install_neuronx_cc_hook()
    partition_name = nc.partition_id_tensor.name if nc.partition_id_tensor else None
    in_names, out_names, out_avals = [], [], []
    for alloc in nc.m.functions[0].allocations:
        if not isinstance(alloc, mybir.MemoryLocationSet):
            continue
        name = alloc.memorylocations[0].name
        if alloc.kind == "ExternalInput":
            if name != partition_name:
                in_names.append(name)
        elif alloc.kind == "ExternalOutput":
            out_names.append(name)
            out_avals.append(
                jax.core.ShapedArray(
                    tuple(alloc.tensor_shape), mybir.dt.np(alloc.dtype)
                )
            )
    all_names = in_names + out_names + ([partition_name] if partition_name else [])

    def _body(*args):
        operands = list(args)
        if partition_name is not None:
            operands.append(partition_id_tensor())
        return tuple(
            _bass_exec_p.bind(
                *operands,
                out_avals=tuple(out_avals),
                in_names=tuple(all_names),
                out_names=tuple(out_names),
                lowering_input_output_aliases=(),
                sim_require_finite=True,
                sim_require_nnan=True,
                nc=nc,
            )
        )

    devices = jax.devices()[:NCORES]
    mesh = Mesh(np.asarray(devices), ("core",))
    sharded = jax.jit(
        shard_map(
            _body,
            mesh=mesh,
            in_specs=(PartitionSpec("core"),) * (len(in_names) + len(out_names)),
            out_specs=(PartitionSpec("core"),) * len(out_names),
            check_rep=False,
        ),
        keep_unused=True,
    )
    sh = NamedSharding(mesh, PartitionSpec("core"))
    dev_zeros = [
        jax.device_put(np.zeros((NCORES * a.shape[0], *a.shape[1:]), a.dtype), sh)
        for a in out_avals
    ]
    runner = (sharded, in_names, out_names, dev_zeros)
    _PROGRAM_CACHE["runner"] = runner
    return runner


def kernel(hidden_states, Wqkv, Wk_up, Wv_up, Wo):
    hidden_states = np.asarray(hidden_states, dtype=np.float32)
    Wqkv = np.asarray(Wqkv, dtype=np.float32)
    Wk_up = np.asarray(Wk_up, dtype=np.float32)
    Wv_up = np.asarray(Wv_up, dtype=np.float32)
    Wo = np.asarray(Wo, dtype=np.float32)

    if "nc" not in _PROGRAM_CACHE:
        _PROGRAM_CACHE["nc"] = _build_program()
    nc = _PROGRAM_CACHE["nc"]

    in_maps = _host_inputs(hidden_states, Wqkv, Wk_up, Wv_up, Wo)

    try:
        from concourse._compat import axon_active

        use_runner = axon_active()
    except Exception:
        use_runner = False

    if use_runner:
        sharded, in_names, out_names, dev_zeros = _get_runner(nc)
        concat_in = [
            np.concatenate([np.asarray(in_maps[c][nm]) for c in range(NCORES)], axis=0)
            for nm in in_names
        ]
        outs = sharded(*concat_in, *dev_zeros)
        y_all = np.asarray(outs[out_names.index("y")]).reshape(NCORES, SC, HID)
        results = [{"y": y_all[c]} for c in range(NCORES)]
    else:
        results = run_bass_kernel_spmd(nc, in_maps, list(range(NCORES))).results
    return _assemble(results)


if __name__ == "__main__":
    rng = np.random.default_rng(0)
    hs = rng.standard_normal((B, S, HID)).astype(np.float32)
    wqkv = rng.standard_normal((HID, NH * D_QK + LORA)).astype(np.float32) * 0.02
    wk = rng.standard_normal((LORA, NKV * D_QK)).astype(np.float32) * 0.04
    wv = rng.standard_normal((LORA, NKV * D_V)).astype(np.float32) * 0.04
    wo = rng.standard_normal((NH * D_V, HID)).astype(np.float32) * 0.02
    y = kernel(hs, wqkv, wk, wv, wo)
    print("kernel output", y.shape, y.dtype, float(np.abs(y).max()))
